# revision 11
# baseline (speedup 1.0000x reference)
"""Monarch / butterfly block-diagonal layer on 8 TRN2 NeuronCores.

Math (reference):
  x:(B,4096) -> out1[b,k,q] = sum_p x[b,k*64+p] * w1[k,q,p]        (64 blocks of 64x64)
  permute (b, k*64+q) -> (b, l=q, r=k)
  out2[b,l,s] = sum_r out1[b,r,l] * w2[l,s,r]                       (64 blocks of 64x64)
  out[b, s*64+l] = out2[b,l,s]

Strategy: pure batch-data-parallel over 8 cores (1024 rows each). All edge
layout conversions (x transpose + bf16 cast, weight packing, output
unpermute + f32 cast) are done host-side in numpy (free). Device pipeline
runs in bf16 (PSUM accumulate stays f32); harness gate is rel_err < 2e-2,
bf16 end-to-end lands at ~3.5e-3.

Variant E dataflow per 256-col batch megatile (tuned against the cost-model
sim, where all DMA transfers serialize on one shared resource and queue
switches cost ~1.8 us init):
  - ALL DMAs ride ONE hwdge queue (SP), ordered [loads(mt+1), stores(mt)]
    so dep-free loads never convoy behind stores.
  - MM1: x tile stationary, 128x128 block-diag w1 tile moving -> PSUM
    (b, (jj,q)); drains scatter to s2[b, q*64+k] bf16 on Act/DVE.
  - T2b: PE transpose (bf16 in -> bf16 PSUM) of s2 128-col slices; 512-el
    bf16 drains to s3 run 2x on DVE.
  - MM2 skewed 2 l-pairs behind T2b so the PE never waits on an s3 drain.
"""

import os
import numpy as np

B_FULL, N = 8192, 4096
NCORES = 8
BC = B_FULL // NCORES       # 1024 rows per core
TILE_B = 256                # megatile batch columns
VARIANT = "E"

# drain assignment knobs (tuned against the cost-model sim):
# f32-source drains (s2, s4) alternate Act/DVE; bf16-source drains (s3)
# prefer DVE (2x 16-bit mode) but spill to Act by pattern.
F32_PAT, F32_ACT = 3, 2     # f32 drains: ctr % PAT < ACT -> Act
S3_PAT, S3_ACT = 8, 0         # s3 drains: ctr % PAT < ACT -> Act
MM2_SKEW = 2

_cache = {}
last_results = None


def _ensure_jax_platform():
    if os.environ.get("JAX_PLATFORMS", "") == "cpu":
        os.environ["JAX_PLATFORMS"] = ""


def _build(bc, tile_b, variant="E", repeat=1):
    import concourse.mybir as mybir
    from concourse import bacc
    from concourse.tile import TileContext
    from concourse.masks import make_identity

    f32 = mybir.dt.float32
    bf16 = mybir.dt.bfloat16
    nmt = bc // tile_b
    nbs = tile_b // 128
    assert nbs == 2 and tile_b == 256

    nc = bacc.Bacc()
    xt = nc.dram_tensor("xt", [128, nmt, 8, 4, tile_b], bf16, kind="ExternalInput")
    w1t = nc.dram_tensor("w1t", [128, 4096], bf16, kind="ExternalInput")
    w2t = nc.dram_tensor("w2t", [128, 4096], bf16, kind="ExternalInput")
    # ot packed: ot[p, mt, h, c, bt], p = lp*64+s, l = 16h + 2c + lp
    ot = nc.dram_tensor("ot", [128, nmt, 4, 8, tile_b], bf16, kind="ExternalOutput")

    with TileContext(nc) as tc:
        with (
            tc.tile_pool(name="wpool", bufs=1) as wpool,
            tc.tile_pool(name="xgp", bufs=16) as xgp,
            tc.tile_pool(name="s2p", bufs=2 * nbs) as s2p,
            tc.tile_pool(name="s3p", bufs=MM2_SKEW + 2) as s3p,
            tc.tile_pool(name="s4p", bufs=8) as s4p,
            tc.tile_pool(name="ps1p", bufs=3, space="PSUM") as ps1p,
            tc.tile_pool(name="ptbp", bufs=3, space="PSUM") as ptbp,
            tc.tile_pool(name="pm2p", bufs=2, space="PSUM") as pm2p,
        ):
            ident = wpool.tile([128, 128], bf16)
            make_identity(nc, ident[:])
            w1s = wpool.tile([128, 4096], bf16)
            w2s = wpool.tile([128, 4096], bf16)
            for wh in range(4):
                nc.sync.dma_start(out=w1s[:, 1024 * wh:1024 * (wh + 1)],
                                  in_=w1t[:, 1024 * wh:1024 * (wh + 1)])
            w2_loaded = [False]

            ctr_f32 = [0]
            ctr_s3 = [0]

            def drain_f32(dst, src):
                if ctr_f32[0] % F32_PAT < F32_ACT:
                    nc.scalar.copy(dst, src)
                else:
                    nc.vector.tensor_copy(out=dst, in_=src)
                ctr_f32[0] += 1

            def drain_s3(dst, src):
                if ctr_s3[0] % S3_PAT < S3_ACT:
                    nc.scalar.copy(dst, src)
                else:
                    nc.vector.tensor_copy(out=dst, in_=src)
                ctr_s3[0] += 1

            def load_mt(mt):
                xg = []
                for g in range(8):
                    t_ = xgp.tile([128, 4, tile_b], bf16, tag="xg")
                    nc.sync.dma_start(out=t_[:], in_=xt[:, mt, g])
                    xg.append(t_)
                return xg

            xg_next = load_mt(0)
            total_mt = repeat * nmt
            for gmt in range(total_mt):
                mt = gmt % nmt
                xg = xg_next
                if not w2_loaded[0]:
                    w2_loaded[0] = True
                    for wh in range(4):
                        nc.sync.dma_start(
                            out=w2s[:, 1024 * wh:1024 * (wh + 1)],
                            in_=w2t[:, 1024 * wh:1024 * (wh + 1)])

                s2_tiles = [
                    s2p.tile([128, 4096], bf16, tag="s2", name="s2t")
                    for _ in range(nbs)
                ]

                # ---- stage 1: fused k-pair matmuls, batch-major out ----
                for bs in range(nbs):
                    for tg in range(8):
                        pm1 = ps1p.tile([128, 4, 128], f32, tag="ps1")
                        for tsub in range(4):
                            t = 4 * tg + tsub
                            nc.tensor.matmul(
                                pm1[:, tsub, :],
                                xg[tg][:, tsub, bs * 128:(bs + 1) * 128],
                                w1s[:, t * 128:(t + 1) * 128],
                            )
                        # psum (b, (tsub, jj, q)) -> s2[b, q*64 + 2t + jj]
                        src = pm1.rearrange("p g (jj q) -> p g jj q", jj=2)
                        dview = s2_tiles[bs].rearrange(
                            "p (q t2 jj) -> p t2 jj q", t2=32, jj=2
                        )
                        drain_f32(dview[:, 4 * tg:4 * tg + 4, :, :], src[:])

                # prefetch next megatile's x while T2b/stage-2 run
                if gmt + 1 < total_mt:
                    xg_next = load_mt((gmt + 1) % nmt)

                # ---- T2b + skewed stage 2 + output drain ----
                s4 = [
                    s4p.tile([128, 8, tile_b], bf16, tag="s4", name="s4t")
                    for _ in range(4)
                ]

                def do_mm2(v, s3, mt=mt, s4=s4):
                    pm2 = pm2p.tile([128, 2, tile_b], f32, tag="pm2")
                    for j2 in range(2):
                        l2 = 2 * v + j2
                        nc.tensor.matmul(
                            pm2[:, j2, :],
                            w2s[:, l2 * 128:(l2 + 1) * 128],
                            s3[:, j2, :],
                        )
                    h, vs = divmod(v, 4)
                    drain_f32(s4[h][:, 2 * vs:2 * vs + 2, :], pm2[:])
                    if vs == 3:
                        nc.sync.dma_start(out=ot[:, mt, h], in_=s4[h][:])

                vq = []
                for v in range(16):
                    ptb = ptbp.tile([128, 2 * nbs, 128], bf16, tag="ptb")
                    for j2 in range(2):
                        l2 = 2 * v + j2
                        for bs in range(nbs):
                            nc.tensor.transpose(
                                ptb[:, j2 * nbs + bs, :],
                                s2_tiles[bs][:, 128 * l2:128 * (l2 + 1)],
                                ident[:],
                            )
                    s3 = s3p.tile([128, 2, tile_b], bf16, tag="s3")
                    drain_s3(
                        s3.rearrange("p j (bs c) -> p j bs c", bs=nbs)[:],
                        ptb.rearrange("p (j bs) c -> p j bs c", j=2)[:],
                    )
                    vq.append((v, s3))
                    if len(vq) > MM2_SKEW:
                        do_mm2(*vq.pop(0))
                for item in vq:
                    do_mm2(*item)

    nc.compile()
    return nc


def _host_prep(x, w1_bfly, w2_bfly):
    """Build per-core device inputs (all numpy, free relative to HW time)."""
    import ml_dtypes
    bf = ml_dtypes.bfloat16

    x = np.asarray(x, dtype=np.float32)
    w1 = np.asarray(w1_bfly, dtype=np.float32)   # (k=64, q=64, p=64)
    w2 = np.asarray(w2_bfly, dtype=np.float32)   # (l=64, s=64, r=64)

    nmt = BC // TILE_B

    # Block-diagonal pair tiles:
    # w1t[half*64+p, t*128 + jj*64 + q] = w1[2t+jj, q, p] if half == jj else 0
    w1t = np.zeros((128, 32, 2, 64), np.float32)
    w1t[0:64, :, 0, :] = w1[0::2].transpose(2, 0, 1)    # (p, t, q)
    w1t[64:128, :, 1, :] = w1[1::2].transpose(2, 0, 1)
    w1t = w1t.reshape(128, 4096).astype(bf)
    # w2t[lp*64+r, l2*128 + lp'*64 + s] = w2[2*l2+lp, s, r] if lp == lp' else 0
    w2t = np.zeros((128, 32, 2, 64), np.float32)
    w2t[0:64, :, 0, :] = w2[0::2].transpose(2, 0, 1)    # (r, l2, s)
    w2t[64:128, :, 1, :] = w2[1::2].transpose(2, 0, 1)
    w2t = w2t.reshape(128, 4096).astype(bf)

    in_maps = []
    for c in range(NCORES):
        shard = x[c * BC:(c + 1) * BC].astype(bf)     # (BC, 4096) bf16
        # xt[p, mt, g, gi, bt] = shard[mt*TILE_B+bt, (4g+gi)*128 + p]
        xtc = np.ascontiguousarray(
            shard.T.reshape(8, 4, 128, nmt, TILE_B).transpose(2, 3, 0, 1, 4)
        )
        in_maps.append({"xt": xtc, "w1t": w1t, "w2t": w2t})
    return in_maps


def _host_post(results):
    """ot[p= lp*64+s, mt, h, c, bt] -> O[b, s*64 + l], l = 16h + 2c + lp."""
    nmt = BC // TILE_B
    out = np.empty((B_FULL, N), np.float32)
    for c, res in enumerate(results):
        ot = np.asarray(res["ot"])                # (128, nmt, 4, 8, TILE_B) bf16
        t = ot.reshape(2, 64, nmt, 4, 8, TILE_B)  # (lp, s, mt, h, cc, bt)
        o = t.transpose(2, 5, 1, 3, 4, 0).reshape(BC, N)
        out[c * BC:(c + 1) * BC] = o.astype(np.float32)
    return out


def kernel(x, w1_bfly, w2_bfly):
    _ensure_jax_platform()
    from concourse.bass_utils import run_bass_kernel_spmd

    global last_results
    if "nc" not in _cache:
        _cache["nc"] = _build(BC, TILE_B, VARIANT)
    nc = _cache["nc"]

    in_maps = _host_prep(x, w1_bfly, w2_bfly)
    trace = os.environ.get("KERNEL_TRACE", "0") == "1"
    res = run_bass_kernel_spmd(
        nc, in_maps, core_ids=list(range(NCORES)), trace=trace
    )
    last_results = res
    return _host_post(res.results)


# revision 12
# speedup vs baseline: 1.4008x; 1.4008x over previous
"""Monarch / butterfly block-diagonal layer on 8 TRN2 NeuronCores.

Math (reference):
  x:(B,4096) -> out1[b,k,q] = sum_p x[b,k*64+p] * w1[k,q,p]        (64 blocks of 64x64)
  permute (b, k*64+q) -> (b, l=q, r=k)
  out2[b,l,s] = sum_r out1[b,r,l] * w2[l,s,r]                       (64 blocks of 64x64)
  out[b, s*64+l] = out2[b,l,s]

Strategy: pure batch-data-parallel over 8 cores (1024 rows each). All edge
layout conversions (x transpose + bf16 cast, weight packing, output
unpermute + f32 cast) are done host-side in numpy (free). Device pipeline
runs in bf16 (PSUM accumulate stays f32); harness gate is rel_err < 2e-2,
bf16 end-to-end lands at ~3.5e-3.

Variant E dataflow per 256-col batch megatile (tuned against the cost-model
sim, where all DMA transfers serialize on one shared resource and queue
switches cost ~1.8 us init):
  - ALL DMAs ride ONE hwdge queue (SP), ordered [loads(mt+1), stores(mt)]
    so dep-free loads never convoy behind stores.
  - MM1: x tile stationary, 128x128 block-diag w1 tile moving -> PSUM
    (b, (jj,q)); drains scatter to s2[b, q*64+k] bf16 on Act/DVE.
  - T2b: PE transpose (bf16 in -> bf16 PSUM) of s2 128-col slices; 512-el
    bf16 drains to s3 run 2x on DVE.
  - MM2 skewed 2 l-pairs behind T2b so the PE never waits on an s3 drain.
"""

import os
import numpy as np

B_FULL, N = 8192, 4096
NCORES = 8
BC = B_FULL // NCORES       # 1024 rows per core
TILE_B = 256                # megatile batch columns
VARIANT = "E2"

# drain assignment knobs (tuned against the cost-model sim):
# f32-source drains (s2, s4) alternate Act/DVE; bf16-source drains (s3)
# prefer DVE (2x 16-bit mode) but spill to Act by pattern.
F32_PAT, F32_ACT = 3, 2  # see LP note     # f32 drains: ctr % PAT < ACT -> Act
S3_PAT, S3_ACT = 8, 0         # s3 drains: ctr % PAT < ACT -> Act
MM2_SKEW = 1

_cache = {}
last_results = None


def _ensure_jax_platform():
    if os.environ.get("JAX_PLATFORMS", "") == "cpu":
        os.environ["JAX_PLATFORMS"] = ""


def _build(bc, tile_b, variant="E", repeat=1):
    import concourse.mybir as mybir
    from concourse import bacc
    from concourse.tile import TileContext
    from concourse.masks import make_identity

    f32 = mybir.dt.float32
    bf16 = mybir.dt.bfloat16
    nmt = bc // tile_b
    nbs = tile_b // 128
    assert nbs == 2 and tile_b == 256

    nc = bacc.Bacc()
    xt = nc.dram_tensor("xt", [128, nmt, 8, 4, tile_b], bf16, kind="ExternalInput")
    w1t = nc.dram_tensor("w1t", [128, 4096], bf16, kind="ExternalInput")
    w2t = nc.dram_tensor("w2t", [128, 4096], bf16, kind="ExternalInput")
    # ot packed: ot[p, mt, h, c, bt], p = lp*64+s, l = 16h + 2c + lp
    ot = nc.dram_tensor("ot", [128, nmt, 4, 8, tile_b], bf16, kind="ExternalOutput")

    with TileContext(nc) as tc:
        with (
            tc.tile_pool(name="wpool", bufs=1) as wpool,
            tc.tile_pool(name="xgp", bufs=16) as xgp,
            tc.tile_pool(name="s2p", bufs=2 * nbs) as s2p,
            tc.tile_pool(name="s3p", bufs=MM2_SKEW + 2) as s3p,
            tc.tile_pool(name="s4p", bufs=8) as s4p,
            tc.tile_pool(name="ps1p", bufs=2, space="PSUM") as ps1p,
            tc.tile_pool(name="ptbp", bufs=2, space="PSUM") as ptbp,
            tc.tile_pool(name="pm2p", bufs=2, space="PSUM") as pm2p,
        ):
            ident = wpool.tile([128, 128], bf16)
            make_identity(nc, ident[:])
            w1s = wpool.tile([128, 4096], bf16)
            w2s = wpool.tile([128, 4096], bf16)
            for wh in range(4):
                nc.sync.dma_start(out=w1s[:, 1024 * wh:1024 * (wh + 1)],
                                  in_=w1t[:, 1024 * wh:1024 * (wh + 1)])
            w2_loaded = [False]

            ctr_f32 = [0]
            ctr_s3 = [0]

            def drain_f32(dst, src):
                if ctr_f32[0] % F32_PAT < F32_ACT:
                    nc.scalar.copy(dst, src)
                else:
                    nc.vector.tensor_copy(out=dst, in_=src)
                ctr_f32[0] += 1

            def drain_s3(dst, src):
                if ctr_s3[0] % S3_PAT < S3_ACT:
                    nc.scalar.copy(dst, src)
                else:
                    nc.vector.tensor_copy(out=dst, in_=src)
                ctr_s3[0] += 1

            def load_mt(mt):
                xg = []
                for g in range(8):
                    t_ = xgp.tile([128, 4, tile_b], bf16, tag="xg")
                    nc.sync.dma_start(out=t_[:], in_=xt[:, mt, g])
                    xg.append(t_)
                return xg

            xg_next = load_mt(0)
            total_mt = repeat * nmt
            for gmt in range(total_mt):
                mt = gmt % nmt
                xg = xg_next
                if not w2_loaded[0]:
                    w2_loaded[0] = True
                    for wh in range(4):
                        nc.sync.dma_start(
                            out=w2s[:, 1024 * wh:1024 * (wh + 1)],
                            in_=w2t[:, 1024 * wh:1024 * (wh + 1)])

                s2_tiles = [
                    s2p.tile([128, 4096], bf16, tag="s2", name="s2t")
                    for _ in range(nbs)
                ]

                # ---- stage 1: fused k-pair matmuls, batch-major out ----
                for bs in range(nbs):
                    for tg4 in range(4):
                        pm1 = ps1p.tile([128, 8, 128], f32, tag="ps1")
                        for u in range(8):
                            t = 8 * tg4 + u
                            g, gi = divmod(t, 4)
                            nc.tensor.matmul(
                                pm1[:, u, :],
                                xg[g][:, gi, bs * 128:(bs + 1) * 128],
                                w1s[:, t * 128:(t + 1) * 128],
                            )
                        # psum (b, (u, jj, q)) -> s2[b, q*64 + (8tg4+u)*2 + jj]
                        srcv = pm1.rearrange("p g (jj q) -> p g jj q", jj=2)
                        dview = s2_tiles[bs].rearrange(
                            "p (q t2 jj) -> p t2 jj q", t2=32, jj=2
                        )
                        drain_f32(dview[:, 8 * tg4:8 * tg4 + 8, :, :], srcv[:])

                # prefetch next megatile's x while T2b/stage-2 run
                if gmt + 1 < total_mt:
                    xg_next = load_mt((gmt + 1) % nmt)

                # ---- T2b + skewed stage 2 + output drain ----
                s4 = [
                    s4p.tile([128, 8, tile_b], bf16, tag="s4", name="s4t")
                    for _ in range(4)
                ]

                def do_mm2(vp, s3, mt=mt, s4=s4):
                    h, hp = divmod(vp, 2)
                    for half in range(2):
                        pm2 = pm2p.tile([128, 2, tile_b], f32, tag="pm2")
                        for j2 in range(2):
                            jv = 2 * half + j2
                            l2 = 4 * vp + jv
                            nc.tensor.matmul(
                                pm2[:, j2, :],
                                w2s[:, l2 * 128:(l2 + 1) * 128],
                                s3[:, jv, :],
                            )
                        drain_f32(
                            s4[h][:, 4 * hp + 2 * half:4 * hp + 2 * half + 2, :],
                            pm2[:])
                    if hp == 1:
                        nc.sync.dma_start(out=ot[:, mt, h], in_=s4[h][:])

                vq = []
                for vp in range(8):
                    ptb = ptbp.tile([128, 4, nbs, 128], bf16, tag="ptb")
                    for jv in range(4):
                        l2 = 4 * vp + jv
                        for bs in range(nbs):
                            nc.tensor.transpose(
                                ptb[:, jv, bs, :],
                                s2_tiles[bs][:, 128 * l2:128 * (l2 + 1)],
                                ident[:],
                            )
                    s3 = s3p.tile([128, 4, tile_b], bf16, tag="s3")
                    drain_s3(
                        s3.rearrange("p j (bs c) -> p j bs c", bs=nbs)[:],
                        ptb[:],
                    )
                    vq.append((vp, s3))
                    if len(vq) > MM2_SKEW:
                        do_mm2(*vq.pop(0))
                for item in vq:
                    do_mm2(*item)

    nc.compile()
    return nc


def _host_prep(x, w1_bfly, w2_bfly):
    """Build per-core device inputs (all numpy, free relative to HW time)."""
    import ml_dtypes
    bf = ml_dtypes.bfloat16

    x = np.asarray(x, dtype=np.float32)
    w1 = np.asarray(w1_bfly, dtype=np.float32)   # (k=64, q=64, p=64)
    w2 = np.asarray(w2_bfly, dtype=np.float32)   # (l=64, s=64, r=64)

    nmt = BC // TILE_B

    # Block-diagonal pair tiles:
    # w1t[half*64+p, t*128 + jj*64 + q] = w1[2t+jj, q, p] if half == jj else 0
    w1t = np.zeros((128, 32, 2, 64), np.float32)
    w1t[0:64, :, 0, :] = w1[0::2].transpose(2, 0, 1)    # (p, t, q)
    w1t[64:128, :, 1, :] = w1[1::2].transpose(2, 0, 1)
    w1t = w1t.reshape(128, 4096).astype(bf)
    # w2t[lp*64+r, l2*128 + lp'*64 + s] = w2[2*l2+lp, s, r] if lp == lp' else 0
    w2t = np.zeros((128, 32, 2, 64), np.float32)
    w2t[0:64, :, 0, :] = w2[0::2].transpose(2, 0, 1)    # (r, l2, s)
    w2t[64:128, :, 1, :] = w2[1::2].transpose(2, 0, 1)
    w2t = w2t.reshape(128, 4096).astype(bf)

    in_maps = []
    for c in range(NCORES):
        shard = x[c * BC:(c + 1) * BC].astype(bf)     # (BC, 4096) bf16
        # xt[p, mt, g, gi, bt] = shard[mt*TILE_B+bt, (4g+gi)*128 + p]
        xtc = np.ascontiguousarray(
            shard.T.reshape(8, 4, 128, nmt, TILE_B).transpose(2, 3, 0, 1, 4)
        )
        in_maps.append({"xt": xtc, "w1t": w1t, "w2t": w2t})
    return in_maps


def _host_post(results):
    """ot[p= lp*64+s, mt, h, c, bt] -> O[b, s*64 + l], l = 16h + 2c + lp."""
    nmt = BC // TILE_B
    out = np.empty((B_FULL, N), np.float32)
    for c, res in enumerate(results):
        ot = np.asarray(res["ot"])                # (128, nmt, 4, 8, TILE_B) bf16
        t = ot.reshape(2, 64, nmt, 4, 8, TILE_B)  # (lp, s, mt, h, cc, bt)
        o = t.transpose(2, 5, 1, 3, 4, 0).reshape(BC, N)
        out[c * BC:(c + 1) * BC] = o.astype(np.float32)
    return out


def kernel(x, w1_bfly, w2_bfly):
    _ensure_jax_platform()
    from concourse.bass_utils import run_bass_kernel_spmd

    global last_results
    if "nc" not in _cache:
        _cache["nc"] = _build(BC, TILE_B, VARIANT)
    nc = _cache["nc"]

    in_maps = _host_prep(x, w1_bfly, w2_bfly)
    trace = os.environ.get("KERNEL_TRACE", "0") == "1"
    res = run_bass_kernel_spmd(
        nc, in_maps, core_ids=list(range(NCORES)), trace=trace
    )
    last_results = res
    return _host_post(res.results)
